# revision 38
# baseline (speedup 1.0000x reference)
"""Trainium2 Bass kernel for nn_ContextEncoder (banded local attention encoder).

Reference computation (B=2, T=2048, D=512, H=8, dh=64, band half-width 32):
  xn   = LayerNorm(x) * g + b
  q    = ((xn @ Wp.T + bp) @ Wq.T + bq) / sqrt(dh)      per-head [B,T,H,dh]
  k, v = xn @ Wk.T + bk, xn @ Wv.T + bv
  s    = banded scores  (|i-j| <= 32), softmax over window
  ctx  = (a @ v_window) @ Wo.T + bo
  gate = sigmoid([x, ctx] @ Wg.T + bg)
  out  = x * (1 - gate) + ctx * gate

Sharding: sequence-parallel, 8 cores = 2 batches x 4 chunks of 512 tokens.
Each core gets its 512-token chunk plus a 32-token halo on each side
(zero-padded at sequence edges; per-core masks kill invalid positions),
computes its 512 output rows fully independently (no collectives), and the
host concatenates.

Algebraic folds done on host:
  - Wp folded into Wq:  q = xn @ (Wq Wp).T * s  -- removes a DxD projection.
  - k-bias dropped: a per-feature constant added to every key shifts each
    query's scores uniformly, which softmax cancels.
  - v-bias folded into bo/bg (ctx picks up exactly +bv after normalization).
  - LN gain/bias folded into weights; gate projections of x and ctx share
    one PSUM accumulation (gate_pre = x@Wg1.T + ctx@(Wg2 Wo).T + const).
  - gate sigmoid computed as 0.5*tanh(z/2)+0.5: tanh lives in the same ACT
    table set as exp, so the kernel performs ZERO mid-kernel table reloads
    (sqrt set loads once at t=0 via a dummy op, exp set once post-LN).

Device pipeline (per core):
  - x tile 0 DMAs first (column halves split across the sync/scalar
    queues; later hi-halves are interleaved between the LN sqrts); LN
    runs per-tile and tile t's PE transposes start as soon as LN(t)
    lands, so the PE starts right after the first LN tile instead of
    waiting for the whole LN phase.
  - Weight DMA issues: wq/wk on sync behind the small x slices, wv and
    the masks on the (slow but otherwise idle) gpsimd SW-DGE queue,
    epilogue operands (xt/wg1/wo/wg2) issued after the LN chain.
  - An ACT sqrt table preload runs at t=0 and an exp preload (pinned
    after the last LN sqrt via a data dependency) right after LN, so
    both ACT_TABLE_LOADs happen while ACT is idle.
  - Scores use the K=128 zero-padded q2 pair layout (contraction
    operands based at partition 64 fault on HW - quadrant-3 xbus).
  - exp on ACT; contiguous pre-duplicated bf16 masks multiply on DVE
    at 2x bf16 rate.
  - AV with V stationary -> ctx feature-major; heads interleaved into
    aligned PE quadrants via a host permutation of Wv/Wo/Wg2.
    Denominators via an all-ones stationary; reciprocal_approx_fast +
    multiply normalizes into the O-projection operand layout.
  - Attention is software-pipelined: block b's scores are emitted before
    block b-1's AV/epilogue so the PE queue never waits on the softmax
    chain.
  - Epilogue: gate = 0.5*tanh(gacc/2)+0.5 (ACT tanh + DVE tensor_scalar),
    bf16 blend chain (2x DVE mode), per-block bf16 stores.
"""

import numpy as np
import ml_dtypes

B, T, D = 2, 2048, 512
H, DH = 8, 64
WCTX = 32
NCORES = 8
CHUNK = 512          # tokens per core
NBLK = CHUNK // 128  # 4 query blocks per core
HALO = CHUNK + 2 * WCTX   # 576 tokens incl. halo
XROWS = 640          # x dram rows: 512 central + 32 left + 32 right + 64 pad
BF16 = ml_dtypes.bfloat16

_CACHE = {}


def _build_program(flags):
    """Builds the single-core Bass/Tile program (shared SPMD across 8 cores).

    flags: (bq_nz, bo_nz, bg_nz) -> emit optional bias adds.
    """
    import concourse.bass as bass
    import concourse.tile as tile
    import concourse.mybir as mybir
    from concourse import bacc

    f32 = mybir.dt.float32
    bf16 = mybir.dt.bfloat16
    AF = mybir.ActivationFunctionType
    ALU = mybir.AluOpType
    bq_nz, bo_nz, bg_nz = flags

    nc = bacc.Bacc(
        "TRN2",
        target_bir_lowering=False,
        debug=False,
        enable_asserts=False,
        num_devices=NCORES,
    )

    x_in = nc.dram_tensor("x", [XROWS, D], bf16, kind="ExternalInput")
    xt_in = nc.dram_tensor("xt", [D, CHUNK], bf16, kind="ExternalInput")
    mA_in = nc.dram_tensor("mA", [128, NBLK, 512], bf16, kind="ExternalInput")
    mB_in = nc.dram_tensor("mB", [64, NBLK, 512], bf16, kind="ExternalInput")
    w_in = {
        n: nc.dram_tensor(n, [D, D], bf16, kind="ExternalInput")
        for n in ["wq", "wk", "wv", "wo", "wg1", "wg2"]
    }
    bqh_in = nc.dram_tensor("bqh", [128, 4], f32, kind="ExternalInput")
    bo_in = nc.dram_tensor("bo", [1, D], f32, kind="ExternalInput")
    bg_in = nc.dram_tensor("bg", [1, D], f32, kind="ExternalInput")
    out_t = nc.dram_tensor("out", [CHUNK, D], bf16, kind="ExternalOutput")

    with tile.TileContext(nc) as tc:
        with (
            tc.tile_pool(name="wpool", bufs=1) as wpool,
            tc.tile_pool(name="apool", bufs=1) as apool,
            tc.tile_pool(name="small", bufs=1) as small,
            tc.tile_pool(name="stats", bufs=12) as stats_pool,
            tc.tile_pool(name="attn", bufs=6) as attn_pool,
            tc.tile_pool(name="rp", bufs=3) as rp_pool,
            tc.tile_pool(name="fin", bufs=3) as fin_pool,
            tc.tile_pool(name="pj", bufs=3, space="PSUM") as pj,
            tc.tile_pool(name="sp0", bufs=2, space="PSUM") as sp0,
            tc.tile_pool(name="sp1", bufs=1, space="PSUM") as sp1,
            tc.tile_pool(name="cp", bufs=1, space="PSUM") as cp,
            tc.tile_pool(name="bcp", bufs=1, space="PSUM") as bcp,
        ):
            # ---- persistent SBUF tensors ----
            x_sb = apool.tile([128, 5, D], bf16, tag="x")
            xn0 = apool.tile([128, 5, D], bf16, tag="xn0")
            xnT = apool.tile([128, 4, HALO], bf16, tag="xnT")
            q2 = apool.tile([128, 4, 2, CHUNK], bf16, tag="q2")
            kT = apool.tile([128, 4, HALO], bf16, tag="kT")
            v_sb = apool.tile([128, 5, D], bf16, tag="v")
            xt_sb = apool.tile([128, 4, CHUNK], bf16, tag="xt")
            mA_sb = apool.tile([128, NBLK, 512], bf16, tag="mA")
            mB_sb = apool.tile([64, NBLK, 512], bf16, tag="mB")
            ctxT = apool.tile([128, 4, NBLK, 128], bf16, tag="ctxT")

            ws = {n: wpool.tile([128, 4, D], bf16, tag=n, name=n) for n in w_in}
            eps_t = small.tile([128, 1], f32, tag="eps")
            preld = small.tile([128, 1], f32, tag="preld")
            ones_sb = small.tile([128, 128], bf16, tag="ones")
            ident = small.tile([128, 128], bf16, tag="ident")

            def wdma(eng, name):
                eng.dma_start(
                    out=ws[name][:],
                    in_=w_in[name][:].rearrange("(c p) d -> p c d", p=128),
                )

            # ---- input DMAs ----
            # x tile 0 goes first, split over two queues, so LN(t0) (and
            # with it the first PE transpose) starts ASAP.  The remaining
            # hi-halves are interleaved between the LN sqrts below so the
            # scalar queue never delays the sqrt chain.
            x_view = x_in[:].rearrange("(c p) d -> p c d", p=128)
            nc.sync.dma_start(out=x_sb[:, 0:1, 0:256], in_=x_view[:, 0:1, 0:256])
            nc.scalar.dma_start(out=x_sb[:, 0:1, 256:512], in_=x_view[:, 0:1, 256:512])
            nc.vector.memset(preld[:], 1.0)
            nc.vector.memset(eps_t[:], 1e-5)
            # sqrt table preload: ACT is idle while x lands; walrus puts the
            # sqrt-set ACT_TABLE_LOAD right before this op, off-critical.
            nc.scalar.activation(out=preld[:], in_=preld[:], func=AF.Sqrt)
            for sl in range(1, 5):
                nc.scalar.dma_start(
                    out=x_sb[:, sl: sl + 1, 256:512],
                    in_=x_view[:, sl: sl + 1, 256:512],
                )
            for sl in range(1, 5):
                nc.sync.dma_start(
                    out=x_sb[:, sl: sl + 1, 0:256],
                    in_=x_view[:, sl: sl + 1, 0:256],
                )
            from concourse.masks import make_identity
            make_identity(nc, ident[:])
            nc.gpsimd.memset(ones_sb[:], 1.0)
            # zero the dead half of each q2 (pair, evenodd) slot once.
            # (contraction operands based at partition 64 fault on HW —
            # quadrant-3 xbus — so scores use the K=128 zero-padded layout)
            for p in range(4):
                nc.gpsimd.memset(q2[64:128, p, 0, :], 0.0)
                nc.gpsimd.memset(q2[0:64, p, 1, :], 0.0)
            # weights: wq/wk on sync after the tiny x-lo issues; wv + masks
            # ride the (slow but idle) gpsimd SW-DGE queue.
            wdma(nc.sync, "wq")
            wdma(nc.sync, "wk")
            wdma(nc.gpsimd, "wv")
            nc.gpsimd.dma_start(out=mA_sb[:], in_=mA_in[:])
            nc.gpsimd.dma_start(out=mB_sb[:], in_=mB_in[:])

            bqh = bo_bc = bg_bc = None
            if bq_nz:
                bqh = small.tile([128, 4], f32, tag="bqh")
                nc.gpsimd.dma_start(out=bqh[:], in_=bqh_in[:])
            if bo_nz:
                bo_bc = small.tile([128, D], f32, tag="bo_bc")
                nc.gpsimd.dma_start(out=bo_bc[:], in_=bo_in[:].to_broadcast([128, D]))
            if bg_nz:
                bg_bc = small.tile([128, D], f32, tag="bg_bc")
                nc.gpsimd.dma_start(out=bg_bc[:], in_=bg_in[:].to_broadcast([128, D]))

            # ---- stage 1+2 fused: software-pipelined LayerNorm -> PE
            # transposes.  The DVE queue is in-order, so tile t+1's
            # bn_stats/bn_aggr are emitted BEFORE tile t's recip/normalize:
            # otherwise the normalize (which waits on the ACT sqrt round
            # trip) blocks the next tile's stats and the PE starves.
            rstd4 = None
            ln_state = {}

            def ln_front(t):
                rows = 128 if t < 4 else 64
                st = stats_pool.tile([128, 6], f32, tag="st")
                mv = stats_pool.tile([128, 2], f32, tag="mv")
                rstd = stats_pool.tile([128, 1], f32, tag="rstd")
                nc.vector.bn_stats(out=st[:rows], in_=x_sb[:rows, t, :])
                nc.vector.bn_aggr(out=mv[:rows], in_=st[:rows])
                nc.scalar.activation(
                    out=rstd[:rows], in_=mv[:rows, 1:2], func=AF.Sqrt,
                    bias=eps_t[:rows], scale=1.0,
                )
                ln_state[t] = (mv, rstd, rows)

            def ln_back(t):
                mv, rstd, rows = ln_state[t]
                nc.vector.reciprocal(out=rstd[:rows], in_=rstd[:rows])
                if t == 0:
                    # tile 0: normalize on DVE - shortest latency to the
                    # first PE transpose (the ACT route adds a hop)
                    nc.vector.tensor_scalar(
                        out=xn0[:rows, t, :], in0=x_sb[:rows, t, :],
                        scalar1=mv[:rows, 0:1], scalar2=rstd[:rows],
                        op0=ALU.subtract, op1=ALU.mult,
                    )
                else:
                    # tiles 1-4: normalize on ACT's free affine (scale/bias
                    # APs): the DVE is the LN cadence bottleneck while ACT
                    # idles between sqrts.  bias = -mu*rstd, one tiny DVE op.
                    nmr = stats_pool.tile([128, 1], f32, tag="nmr")
                    nc.vector.tensor_scalar(
                        out=nmr[:rows], in0=mv[:rows, 0:1],
                        scalar1=rstd[:rows], scalar2=-1.0,
                        op0=ALU.mult, op1=ALU.mult,
                    )
                    nc.scalar.activation(
                        out=xn0[:rows, t, :], in_=x_sb[:rows, t, :],
                        func=AF.Identity, bias=nmr[:rows], scale=rstd[:rows],
                    )
                if t < 4:
                    tp = pj.tile([128, 4, 128], bf16, tag="pj", name=f"tp{t}")
                    for j in range(4):
                        nc.tensor.transpose(
                            tp[:, j, :],
                            xn0[:, t, 128 * j: 128 * (j + 1)],
                            ident[:],
                        )
                    nc.vector.tensor_copy(
                        out=xnT[:, :, 32 + 128 * t: 32 + 128 * (t + 1)],
                        in_=tp[:],
                    )
                else:
                    # halo tokens: x rows [512:544] -> halo 0..32,
                    # [544:576] -> halo 544..576
                    tp = pj.tile([128, 4, 128], bf16, tag="pj", name="tph")
                    for j in range(4):
                        nc.tensor.transpose(
                            tp[:, j, 0:64],
                            xn0[0:64, 4, 128 * j: 128 * (j + 1)],
                            ident[0:64, 0:64],
                        )
                    nc.vector.tensor_copy(
                        out=xnT[:, :, 0:32], in_=tp[:, :, 0:32]
                    )
                    nc.vector.tensor_copy(
                        out=xnT[:, :, 544:576], in_=tp[:, :, 32:64]
                    )

            for t in [0, 1, 2, 3, 4]:
                ln_front(t)
                if t >= 1:
                    ln_back(t - 1)
            ln_back(4)
            rstd4 = ln_state[4][1]

            # exp/tanh table preload on the now-idle ACT; the rstd4 input
            # pins it AFTER the last LN sqrt in the ACT stream.  All later
            # ACT ops (Copy, Exp, Tanh) live in this one set -> no reloads.
            nc.scalar.activation(out=preld[0:1], in_=rstd4[0:1], func=AF.Exp, scale=0.0)

            # late inputs: epilogue operands, issued after the LN chain
            nc.scalar.dma_start(
                out=xt_sb[:], in_=xt_in[:].rearrange("(c p) d -> p c d", p=128)
            )
            wdma(nc.scalar, "wg1")
            wdma(nc.sync, "wo")
            wdma(nc.sync, "wg2")

            # ---- stage 3: projections ----
            # q: folded single projection, written into the zero-padded pair
            # layout (head-even rows -> partitions 0-63, head-odd -> 64-127)
            for j in range(4):
                ps = pj.tile([128, 512], f32, tag="pj")
                for c in range(4):
                    nc.tensor.matmul(
                        ps[:], ws["wq"][:, c, 128 * j: 128 * (j + 1)],
                        xnT[:, c, 32: 32 + CHUNK],
                        start=(c == 0), stop=(c == 3),
                    )
                if bq_nz:
                    nc.vector.tensor_scalar(
                        out=q2[0:64, j, 0, :], in0=ps[0:64],
                        scalar1=bqh[0:64, j: j + 1], scalar2=None, op0=ALU.add,
                    )
                    nc.vector.tensor_scalar(
                        out=q2[64:128, j, 1, :], in0=ps[64:128],
                        scalar1=bqh[64:128, j: j + 1], scalar2=None, op0=ALU.add,
                    )
                else:
                    nc.scalar.activation(
                        out=q2[0:64, j, 0, :], in_=ps[0:64], func=AF.Copy
                    )
                    nc.scalar.activation(
                        out=q2[64:128, j, 1, :], in_=ps[64:128], func=AF.Copy
                    )
            # kT[d, w] = Wk_eff @ xnT  (all 576 halo tokens; k-bias dropped)
            for j in range(4):
                ps = pj.tile([128, 512], f32, tag="pj")
                ps2 = pj.tile([128, 512], f32, tag="pj")
                for c in range(4):
                    nc.tensor.matmul(
                        ps[:], ws["wk"][:, c, 128 * j: 128 * (j + 1)],
                        xnT[:, c, 0:512],
                        start=(c == 0), stop=(c == 3),
                    )
                for c in range(4):
                    nc.tensor.matmul(
                        ps2[:, 0:64], ws["wk"][:, c, 128 * j: 128 * (j + 1)],
                        xnT[:, c, 512:576],
                        start=(c == 0), stop=(c == 3),
                    )
                nc.scalar.activation(out=kT[:, j, 0:512], in_=ps[:], func=AF.Copy)
                nc.scalar.activation(
                    out=kT[:, j, 512:576], in_=ps2[:, 0:64], func=AF.Copy
                )
            # v token-major (feature-permuted Wv; bias folded into bo/bg)
            for t in range(5):
                rows = 128 if t < 4 else 64
                ps = pj.tile([128, 512], f32, tag="pj")
                for c in range(4):
                    nc.tensor.matmul(
                        ps[:rows], xnT[:, c, 128 * t: 128 * t + rows],
                        ws["wv"][:, c, :],
                        start=(c == 0), stop=(c == 3),
                    )
                nc.scalar.activation(
                    out=v_sb[:rows, t, :], in_=ps[:rows], func=AF.Copy
                )

            # ---- stage 4: software-pipelined banded attention ----
            # stage st emits scores+exp for block st, then the AV/normalize
            # consumers for block st-1, then block st's masks, then block
            # st-1's epilogue.
            a_tiles = {}
            for st in range(NBLK + 1):
                if st < NBLK:
                    b = st
                    for g in range(2):
                        s0 = sp0.tile([128, 2, 256], f32, tag="s0")
                        s1 = sp1.tile([64, 2, 256], f32, tag="s1")
                        for pr in range(2):
                            p = 2 * g + pr
                            q_ap = q2[:, p, :, 128 * b: 128 * (b + 1)]
                            nc.tensor.matmul(
                                s0[:, pr, :],
                                kT[:, p, 128 * b: 128 * b + 128],
                                q_ap, start=True, stop=True,
                            )
                            nc.tensor.matmul(
                                s1[:, pr, :],
                                kT[:, p, 128 * b + 128: 128 * b + 192],
                                q_ap, start=True, stop=True,
                            )
                        a0 = attn_pool.tile([128, 2, 256], bf16, tag="a0")
                        a1 = attn_pool.tile([64, 2, 256], bf16, tag="a1")
                        # s1 first: its single PSUM buffer is the next
                        # score-matmul's dependency
                        nc.scalar.activation(
                            out=a1[:].rearrange("p a b -> p (a b)"),
                            in_=s1[:].rearrange("p a b -> p (a b)"), func=AF.Exp,
                        )
                        nc.scalar.activation(
                            out=a0[:].rearrange("p a b -> p (a b)"),
                            in_=s0[:].rearrange("p a b -> p (a b)"), func=AF.Exp,
                        )
                        a_tiles[(b, g)] = (a0, a1)
                if st >= 1:
                    bp_ = st - 1
                    cps = cp.tile([128, 4, 128], f32, tag="cps")
                    for g in range(2):
                        a0, a1 = a_tiles[(bp_, g)]
                        a0f = a0[:].rearrange("p a b -> p (a b)")
                        a1f = a1[:].rearrange("p a b -> p (a b)")
                        # denominators broadcast to every partition by an
                        # all-ones stationary
                        bc = bcp.tile([128, 512], f32, tag="bc")
                        nc.tensor.matmul(
                            bc[:], ones_sb[:], a0f, start=True, stop=False,
                        )
                        nc.tensor.matmul(
                            bc[:], ones_sb[0:64, :], a1f, start=False, stop=True,
                        )
                        # AV: V stationary -> ctx feature-major (head h in
                        # feature tile h%4, partition half h//4 = g)
                        po = 64 * g
                        for hh in range(4):
                            nc.tensor.matmul(
                                cps[po: po + 64, hh, :],
                                v_sb[:, bp_, 128 * hh + po: 128 * hh + po + 64],
                                a0[:, hh >> 1, 128 * (hh & 1): 128 * (hh & 1) + 128],
                                start=True, stop=False,
                            )
                            nc.tensor.matmul(
                                cps[po: po + 64, hh, :],
                                v_sb[0:64, bp_ + 1, 128 * hh + po: 128 * hh + po + 64],
                                a1[:, hh >> 1, 128 * (hh & 1): 128 * (hh & 1) + 128],
                                start=False, stop=True,
                            )
                        rbc = rp_pool.tile([64, 512], f32, tag="rbc")
                        nc.vector.reciprocal_approx_fast(
                            out=rbc[:], in_=bc[po: po + 64, :]
                        )
                        nc.vector.tensor_mul(
                            out=ctxT[po: po + 64, :, bp_, :],
                            in0=cps[po: po + 64, :, :],
                            in1=rbc[:].rearrange("p (a q) -> p a q", q=128),
                        )
                        del a_tiles[(bp_, g)]
                if st < NBLK:
                    b = st
                    for g in range(2):
                        a0, a1 = a_tiles[(b, g)]
                        nc.vector.tensor_mul(
                            out=a1[:].rearrange("p a b -> p (a b)"),
                            in0=a1[:].rearrange("p a b -> p (a b)"),
                            in1=mB_sb[:, b, :],
                        )
                        nc.vector.tensor_mul(
                            out=a0[:].rearrange("p a b -> p (a b)"),
                            in0=a0[:].rearrange("p a b -> p (a b)"),
                            in1=mA_sb[:, b, :],
                        )
                if st >= 1:
                    b = st - 1
                    # ---- epilogue for block b: gate (x and ctx parts share
                    # one PSUM accumulation), O-proj, tanh-gate, blend, store
                    gacc = pj.tile([128, 512], f32, tag="pj", name=f"gacc{b}")
                    for c in range(4):
                        nc.tensor.matmul(
                            gacc[:], xt_sb[:, c, 128 * b: 128 * (b + 1)],
                            ws["wg1"][:, c, :],
                            start=(c == 0), stop=False,
                        )
                    for c in range(4):
                        nc.tensor.matmul(
                            gacc[:], ctxT[:, c, b, :], ws["wg2"][:, c, :],
                            start=False, stop=(c == 3),
                        )
                    ops = pj.tile([128, 512], f32, tag="pj")
                    for c in range(4):
                        nc.tensor.matmul(
                            ops[:], ctxT[:, c, b, :], ws["wo"][:, c, :],
                            start=(c == 0), stop=(c == 3),
                        )
                    diff = fin_pool.tile([128, 512], bf16, tag="diff")
                    th = fin_pool.tile([128, 512], bf16, tag="th")
                    gate = fin_pool.tile([128, 512], bf16, tag="gate")
                    outs = fin_pool.tile([128, 512], bf16, tag="outs")
                    if bo_nz:
                        nc.vector.tensor_add(out=ops[:], in0=ops[:], in1=bo_bc[:])
                    nc.vector.tensor_sub(out=diff[:], in0=ops[:], in1=x_sb[:, b, :])
                    if bg_nz:
                        nc.vector.tensor_add(out=gacc[:], in0=gacc[:], in1=bg_bc[:])
                    # sigmoid(z) = 0.5*tanh(z/2) + 0.5  (tanh shares the exp
                    # table set -> no ACT table reload)
                    nc.scalar.activation(
                        out=th[:], in_=gacc[:], func=AF.Tanh, scale=0.5,
                    )
                    nc.vector.tensor_scalar(
                        out=gate[:], in0=th[:],
                        scalar1=0.5, scalar2=0.5, op0=ALU.mult, op1=ALU.add,
                    )
                    # out = x + gate * (o - x); bf16 chain runs DVE at 2x
                    nc.vector.tensor_mul(out=diff[:], in0=diff[:], in1=gate[:])
                    nc.vector.tensor_add(out=outs[:], in0=diff[:], in1=x_sb[:, b, :])
                    nc.sync.dma_start(
                        out=out_t[:].rearrange("(c p) d -> p c d", p=128)[:, b, :],
                        in_=outs[:],
                    )
    nc.compile()
    return nc


def _host_prep(inputs):
    """Fold LN gain/bias + scale + Wp + bv into weights, build per-core maps."""
    x = np.asarray(inputs["token_embeds"], np.float32)
    g = np.asarray(inputs["ln_g"], np.float32)
    lb = np.asarray(inputs["ln_b"], np.float32)
    Wp = np.asarray(inputs["Wp"], np.float32)
    Wq = np.asarray(inputs["Wq"], np.float32)
    Wk = np.asarray(inputs["Wk"], np.float32)
    Wv = np.asarray(inputs["Wv"], np.float32)
    Wo = np.asarray(inputs["Wo"], np.float32)
    Wg = np.asarray(inputs["Wg"], np.float32)
    bp = np.asarray(inputs["bp"], np.float32)
    bq = np.asarray(inputs["bq"], np.float32)
    bv = np.asarray(inputs["bv"], np.float32)
    bo = np.asarray(inputs["bo"], np.float32)
    bg = np.asarray(inputs["bg"], np.float32)

    scale = 1.0 / np.sqrt(np.float32(DH))
    # feature permutation for ctx: head h features -> tile h%4, half h//4
    perm = np.zeros(D, np.int64)
    for h in range(H):
        c, gg = h % 4, h // 4
        perm[128 * c + 64 * gg: 128 * c + 64 * gg + 64] = np.arange(
            64 * h, 64 * h + 64
        )

    Wpq = (Wq @ Wp) * scale                       # folded q projection
    wq = np.ascontiguousarray((Wpq * g[None, :]).T).astype(BF16)
    wk = np.ascontiguousarray((Wk * g[None, :]).T).astype(BF16)
    wv_p = (Wv * g[None, :])[perm, :]             # permuted output features
    wv = np.ascontiguousarray(wv_p.T).astype(BF16)
    wo = np.ascontiguousarray(Wo[:, perm].T).astype(BF16)
    wg1 = np.ascontiguousarray(Wg[:, :D].T).astype(BF16)
    # reference gates on ctx AFTER the O-projection; fold Wo into Wg2 so the
    # gate matmul can consume pre-projection (permuted) ctx directly
    Wg2o = Wg[:, D:] @ Wo
    wg2 = np.ascontiguousarray(Wg2o[:, perm].T).astype(BF16)

    bq_eff = (Wq @ (Wp @ lb + bp) + bq) * scale
    bv_eff = Wv @ lb + bv
    # device ctx omits the v-bias; it re-enters as a constant through both
    # the O-projection and the folded gate projection
    bo_eff = Wo @ bv_eff + bo
    bg_eff = Wg[:, D:] @ bo_eff + bg

    bqh = np.ascontiguousarray(bq_eff.reshape(4, 128).T).astype(np.float32)
    flags = (
        bool(np.any(bq_eff != 0)),
        bool(np.any(bo_eff != 0)),
        bool(np.any(bg_eff != 0)),
    )

    in_maps = []
    for core in range(NCORES):
        bi, ci = core // 4, core % 4
        s = ci * CHUNK
        xr = np.zeros((XROWS, D), BF16)
        xr[0:CHUNK] = x[bi, s: s + CHUNK]
        if s - WCTX >= 0:
            xr[CHUNK: CHUNK + WCTX] = x[bi, s - WCTX: s]
        if s + CHUNK + WCTX <= T:
            xr[CHUNK + WCTX: CHUNK + 2 * WCTX] = x[bi, s + CHUNK: s + CHUNK + WCTX]
        xt = np.ascontiguousarray(x[bi, s: s + CHUNK].T).astype(BF16)

        # mask[b, rr, cc]: query r=128b+rr (local), key halo pos j=128b+cc;
        # duplicated 4x along columns (pair x evenodd) so the on-device
        # multiply is a contiguous 2D bf16 op
        rr = np.arange(128)[:, None]
        cc = np.arange(192)[None, :]
        m = np.zeros((NBLK, 128, 192), np.float32)
        for qb in range(NBLK):
            band = (cc - rr >= 0) & (cc - rr <= 2 * WCTX)
            gkey = s + 128 * qb + cc - WCTX + 0 * rr
            m[qb] = (band & (gkey >= 0) & (gkey < T)).astype(np.float32)
        mA = np.ascontiguousarray(
            np.tile(m[:, :, :128].transpose(2, 0, 1), (1, 1, 4))
        ).astype(BF16)
        mB = np.ascontiguousarray(
            np.tile(m[:, :, 128:].transpose(2, 0, 1), (1, 1, 4))
        ).astype(BF16)

        in_maps.append({
            "x": xr, "xt": xt, "mA": mA, "mB": mB,
            "wq": wq, "wk": wk, "wv": wv, "wo": wo,
            "wg1": wg1, "wg2": wg2,
            "bqh": bqh,
            "bo": bo_eff.reshape(1, D).astype(np.float32),
            "bg": bg_eff.reshape(1, D).astype(np.float32),
        })
    return in_maps, flags


def _run(inputs, trace=False):
    from concourse.bass_utils import run_bass_kernel_spmd

    in_maps, flags = _host_prep(inputs)
    if flags not in _CACHE:
        _CACHE[flags] = _build_program(flags)
    nc = _CACHE[flags]
    res = run_bass_kernel_spmd(nc, in_maps, list(range(NCORES)), trace=trace)
    out = np.zeros((B, T, D), np.float32)
    for core in range(NCORES):
        bi, ci = core // 4, core % 4
        out[bi, ci * CHUNK: (ci + 1) * CHUNK] = np.asarray(
            res.results[core]["out"], dtype=np.float32
        )
    return out, res


def kernel(**inputs):
    out, _ = _run(inputs, trace=False)
    return out


# revision 39
# speedup vs baseline: 1.0480x; 1.0480x over previous
"""Trainium2 Bass kernel for nn_ContextEncoder (banded local attention encoder).

Reference computation (B=2, T=2048, D=512, H=8, dh=64, band half-width 32):
  xn   = LayerNorm(x) * g + b
  q    = ((xn @ Wp.T + bp) @ Wq.T + bq) / sqrt(dh)      per-head [B,T,H,dh]
  k, v = xn @ Wk.T + bk, xn @ Wv.T + bv
  s    = banded scores  (|i-j| <= 32), softmax over window
  ctx  = (a @ v_window) @ Wo.T + bo
  gate = sigmoid([x, ctx] @ Wg.T + bg)
  out  = x * (1 - gate) + ctx * gate

Sharding: sequence-parallel, 8 cores = 2 batches x 4 chunks of 512 tokens.
Each core gets its 512-token chunk plus a 32-token halo on each side
(zero-padded at sequence edges; per-core masks kill invalid positions),
computes its 512 output rows fully independently (no collectives), and the
host concatenates.

Algebraic folds done on host:
  - Wp folded into Wq:  q = xn @ (Wq Wp).T * s  -- removes a DxD projection.
  - k-bias dropped: a per-feature constant added to every key shifts each
    query's scores uniformly, which softmax cancels.
  - v-bias folded into bo/bg (ctx picks up exactly +bv after normalization).
  - LN gain/bias folded into weights; gate projections of x and ctx share
    one PSUM accumulation (gate_pre = x@Wg1.T + ctx@(Wg2 Wo).T + const).
  - gate sigmoid computed as 0.5*tanh(z/2)+0.5: tanh lives in the same ACT
    table set as exp, so the kernel performs ZERO mid-kernel table reloads
    (sqrt set loads once at t=0 via a dummy op, exp set once post-LN).

Device pipeline (per core):
  - x tile 0 DMAs first (column halves split across the sync/scalar
    queues; later hi-halves are interleaved between the LN sqrts); LN
    runs per-tile and tile t's PE transposes start as soon as LN(t)
    lands, so the PE starts right after the first LN tile instead of
    waiting for the whole LN phase.
  - Weight DMA issues: wq/wk on sync behind the small x slices, wv and
    the masks on the (slow but otherwise idle) gpsimd SW-DGE queue,
    epilogue operands (xt/wg1/wo/wg2) issued after the LN chain.
  - An ACT sqrt table preload runs at t=0 and an exp preload (pinned
    after the last LN sqrt via a data dependency) right after LN, so
    both ACT_TABLE_LOADs happen while ACT is idle.
  - Scores use the K=128 zero-padded q2 pair layout (contraction
    operands based at partition 64 fault on HW - quadrant-3 xbus).
  - exp on ACT; contiguous pre-duplicated bf16 masks multiply on DVE
    at 2x bf16 rate.
  - AV with V stationary -> ctx feature-major; heads interleaved into
    aligned PE quadrants via a host permutation of Wv/Wo/Wg2.
    Denominators via an all-ones stationary; reciprocal_approx_fast +
    multiply normalizes into the O-projection operand layout.
  - Attention is software-pipelined: block b's scores are emitted before
    block b-1's AV/epilogue so the PE queue never waits on the softmax
    chain.
  - Epilogue: gate = 0.5*tanh(gacc/2)+0.5 (ACT tanh + DVE tensor_scalar),
    bf16 blend chain (2x DVE mode), per-block bf16 stores.
"""

import numpy as np
import ml_dtypes

B, T, D = 2, 2048, 512
H, DH = 8, 64
WCTX = 32
NCORES = 8
CHUNK = 512          # tokens per core
NBLK = CHUNK // 128  # 4 query blocks per core
HALO = CHUNK + 2 * WCTX   # 576 tokens incl. halo
XROWS = 640          # x dram rows: 512 central + 32 left + 32 right + 64 pad
BF16 = ml_dtypes.bfloat16

_CACHE = {}


def _build_program(flags):
    """Builds the single-core Bass/Tile program (shared SPMD across 8 cores).

    flags: (bq_nz, bo_nz, bg_nz) -> emit optional bias adds.
    """
    import concourse.bass as bass
    import concourse.tile as tile
    import concourse.mybir as mybir
    from concourse import bacc

    f32 = mybir.dt.float32
    bf16 = mybir.dt.bfloat16
    AF = mybir.ActivationFunctionType
    ALU = mybir.AluOpType
    bq_nz, bo_nz, bg_nz = flags

    nc = bacc.Bacc(
        "TRN2",
        target_bir_lowering=False,
        debug=False,
        enable_asserts=False,
        num_devices=NCORES,
    )

    x_in = nc.dram_tensor("x", [XROWS, D], bf16, kind="ExternalInput")
    xt_in = nc.dram_tensor("xt", [D, CHUNK], bf16, kind="ExternalInput")
    mA_in = nc.dram_tensor("mA", [128, NBLK, 512], bf16, kind="ExternalInput")
    mB_in = nc.dram_tensor("mB", [64, NBLK, 512], bf16, kind="ExternalInput")
    w_in = {
        n: nc.dram_tensor(n, [D, D], bf16, kind="ExternalInput")
        for n in ["wq", "wk", "wv", "wo", "wg1", "wg2"]
    }
    bqh_in = nc.dram_tensor("bqh", [128, 4], f32, kind="ExternalInput")
    bo_in = nc.dram_tensor("bo", [1, D], f32, kind="ExternalInput")
    bg_in = nc.dram_tensor("bg", [1, D], f32, kind="ExternalInput")
    out_t = nc.dram_tensor("out", [CHUNK, D], bf16, kind="ExternalOutput")

    with tile.TileContext(nc) as tc:
        with (
            tc.tile_pool(name="wpool", bufs=1) as wpool,
            tc.tile_pool(name="apool", bufs=1) as apool,
            tc.tile_pool(name="small", bufs=1) as small,
            tc.tile_pool(name="stats", bufs=12) as stats_pool,
            tc.tile_pool(name="attn", bufs=6) as attn_pool,
            tc.tile_pool(name="rp", bufs=3) as rp_pool,
            tc.tile_pool(name="fin", bufs=3) as fin_pool,
            tc.tile_pool(name="pj", bufs=3, space="PSUM") as pj,
            tc.tile_pool(name="sp0", bufs=2, space="PSUM") as sp0,
            tc.tile_pool(name="sp1", bufs=1, space="PSUM") as sp1,
            tc.tile_pool(name="cp", bufs=1, space="PSUM") as cp,
            tc.tile_pool(name="bcp", bufs=1, space="PSUM") as bcp,
        ):
            # ---- persistent SBUF tensors ----
            x_sb = apool.tile([128, 5, D], bf16, tag="x")
            xn0 = apool.tile([128, 5, D], bf16, tag="xn0")
            xnT = apool.tile([128, 4, HALO], bf16, tag="xnT")
            q2 = apool.tile([128, 4, 2, CHUNK], bf16, tag="q2")
            kT = apool.tile([128, 4, HALO], bf16, tag="kT")
            v_sb = apool.tile([128, 5, D], bf16, tag="v")
            xt_sb = apool.tile([128, 4, CHUNK], bf16, tag="xt")
            mA_sb = apool.tile([128, NBLK, 512], bf16, tag="mA")
            mB_sb = apool.tile([64, NBLK, 512], bf16, tag="mB")
            ctxT = apool.tile([128, 4, NBLK, 128], bf16, tag="ctxT")

            ws = {n: wpool.tile([128, 4, D], bf16, tag=n, name=n) for n in w_in}
            eps_t = small.tile([128, 1], f32, tag="eps")
            preld = small.tile([128, 1], f32, tag="preld")
            ones_sb = small.tile([128, 128], bf16, tag="ones")
            ident = small.tile([128, 128], bf16, tag="ident")

            def wdma(eng, name):
                eng.dma_start(
                    out=ws[name][:],
                    in_=w_in[name][:].rearrange("(c p) d -> p c d", p=128),
                )

            # ---- input DMAs ----
            # x tile 0 goes first, split over two queues, so LN(t0) (and
            # with it the first PE transpose) starts ASAP.  The remaining
            # hi-halves are interleaved between the LN sqrts below so the
            # scalar queue never delays the sqrt chain.
            x_view = x_in[:].rearrange("(c p) d -> p c d", p=128)
            nc.sync.dma_start(out=x_sb[:, 0:1, 0:256], in_=x_view[:, 0:1, 0:256])
            nc.scalar.dma_start(out=x_sb[:, 0:1, 256:512], in_=x_view[:, 0:1, 256:512])
            nc.vector.memset(preld[:], 1.0)
            nc.vector.memset(eps_t[:], 1e-5)
            # sqrt table preload: ACT is idle while x lands; walrus puts the
            # sqrt-set ACT_TABLE_LOAD right before this op, off-critical.
            nc.scalar.activation(out=preld[:], in_=preld[:], func=AF.Sqrt)
            for sl in range(1, 5):
                nc.scalar.dma_start(
                    out=x_sb[:, sl: sl + 1, 256:512],
                    in_=x_view[:, sl: sl + 1, 256:512],
                )
            for sl in range(1, 5):
                nc.sync.dma_start(
                    out=x_sb[:, sl: sl + 1, 0:256],
                    in_=x_view[:, sl: sl + 1, 0:256],
                )
            from concourse.masks import make_identity
            make_identity(nc, ident[:])
            nc.gpsimd.memset(ones_sb[:], 1.0)
            # zero the dead half of each q2 (pair, evenodd) slot once.
            # (contraction operands based at partition 64 fault on HW —
            # quadrant-3 xbus — so scores use the K=128 zero-padded layout)
            for p in range(4):
                nc.gpsimd.memset(q2[64:128, p, 0, :], 0.0)
                nc.gpsimd.memset(q2[0:64, p, 1, :], 0.0)
            # weights: wq/wk on sync after the tiny x-lo issues; wv + masks
            # ride the (slow but idle) gpsimd SW-DGE queue.
            wdma(nc.sync, "wq")
            wdma(nc.sync, "wk")
            wdma(nc.gpsimd, "wv")
            nc.gpsimd.dma_start(out=mA_sb[:], in_=mA_in[:])
            nc.gpsimd.dma_start(out=mB_sb[:], in_=mB_in[:])

            bqh = bo_bc = bg_bc = None
            if bq_nz:
                bqh = small.tile([128, 4], f32, tag="bqh")
                nc.gpsimd.dma_start(out=bqh[:], in_=bqh_in[:])
            if bo_nz:
                bo_bc = small.tile([128, D], f32, tag="bo_bc")
                nc.gpsimd.dma_start(out=bo_bc[:], in_=bo_in[:].to_broadcast([128, D]))
            if bg_nz:
                bg_bc = small.tile([128, D], f32, tag="bg_bc")
                nc.gpsimd.dma_start(out=bg_bc[:], in_=bg_in[:].to_broadcast([128, D]))

            # ---- stage 1+2 fused: software-pipelined LayerNorm -> PE
            # transposes.  The DVE queue is in-order, so tile t+1's
            # bn_stats/bn_aggr are emitted BEFORE tile t's recip/normalize:
            # otherwise the normalize (which waits on the ACT sqrt round
            # trip) blocks the next tile's stats and the PE starves.
            rstd4 = None
            ln_state = {}

            def ln_front(t):
                rows = 128 if t < 4 else 64
                st = stats_pool.tile([128, 6], f32, tag="st")
                mv = stats_pool.tile([128, 2], f32, tag="mv")
                rstd = stats_pool.tile([128, 1], f32, tag="rstd")
                nc.vector.bn_stats(out=st[:rows], in_=x_sb[:rows, t, :])
                nc.vector.bn_aggr(out=mv[:rows], in_=st[:rows])
                nc.scalar.activation(
                    out=rstd[:rows], in_=mv[:rows, 1:2], func=AF.Sqrt,
                    bias=eps_t[:rows], scale=1.0,
                )
                ln_state[t] = (mv, rstd, rows)

            def ln_back(t):
                mv, rstd, rows = ln_state[t]
                nc.vector.reciprocal(out=rstd[:rows], in_=rstd[:rows])
                nc.vector.tensor_scalar(
                    out=xn0[:rows, t, :], in0=x_sb[:rows, t, :],
                    scalar1=mv[:rows, 0:1], scalar2=rstd[:rows],
                    op0=ALU.subtract, op1=ALU.mult,
                )
                if t < 4:
                    tp = pj.tile([128, 4, 128], bf16, tag="pj", name=f"tp{t}")
                    for j in range(4):
                        nc.tensor.transpose(
                            tp[:, j, :],
                            xn0[:, t, 128 * j: 128 * (j + 1)],
                            ident[:],
                        )
                    nc.vector.tensor_copy(
                        out=xnT[:, :, 32 + 128 * t: 32 + 128 * (t + 1)],
                        in_=tp[:],
                    )
                else:
                    # halo tokens: x rows [512:544] -> halo 0..32,
                    # [544:576] -> halo 544..576
                    tp = pj.tile([128, 4, 128], bf16, tag="pj", name="tph")
                    for j in range(4):
                        nc.tensor.transpose(
                            tp[:, j, 0:64],
                            xn0[0:64, 4, 128 * j: 128 * (j + 1)],
                            ident[0:64, 0:64],
                        )
                    nc.vector.tensor_copy(
                        out=xnT[:, :, 0:32], in_=tp[:, :, 0:32]
                    )
                    nc.vector.tensor_copy(
                        out=xnT[:, :, 544:576], in_=tp[:, :, 32:64]
                    )

            for t in [0, 1, 2, 3, 4]:
                ln_front(t)
                if t >= 1:
                    ln_back(t - 1)
            ln_back(4)
            rstd4 = ln_state[4][1]

            # exp/tanh table preload on the now-idle ACT; the rstd4 input
            # pins it AFTER the last LN sqrt in the ACT stream.  All later
            # ACT ops (Copy, Exp, Tanh) live in this one set -> no reloads.
            nc.scalar.activation(out=preld[0:1], in_=rstd4[0:1], func=AF.Exp, scale=0.0)

            # late inputs: epilogue operands, issued after the LN chain
            nc.scalar.dma_start(
                out=xt_sb[:], in_=xt_in[:].rearrange("(c p) d -> p c d", p=128)
            )
            wdma(nc.scalar, "wg1")
            wdma(nc.sync, "wo")
            wdma(nc.sync, "wg2")

            # ---- stage 3: projections ----
            # q: folded single projection, written into the zero-padded pair
            # layout (head-even rows -> partitions 0-63, head-odd -> 64-127)
            for j in range(4):
                ps = pj.tile([128, 512], f32, tag="pj")
                for c in range(4):
                    nc.tensor.matmul(
                        ps[:], ws["wq"][:, c, 128 * j: 128 * (j + 1)],
                        xnT[:, c, 32: 32 + CHUNK],
                        start=(c == 0), stop=(c == 3),
                    )
                if bq_nz:
                    nc.vector.tensor_scalar(
                        out=q2[0:64, j, 0, :], in0=ps[0:64],
                        scalar1=bqh[0:64, j: j + 1], scalar2=None, op0=ALU.add,
                    )
                    nc.vector.tensor_scalar(
                        out=q2[64:128, j, 1, :], in0=ps[64:128],
                        scalar1=bqh[64:128, j: j + 1], scalar2=None, op0=ALU.add,
                    )
                else:
                    nc.scalar.activation(
                        out=q2[0:64, j, 0, :], in_=ps[0:64], func=AF.Copy
                    )
                    nc.scalar.activation(
                        out=q2[64:128, j, 1, :], in_=ps[64:128], func=AF.Copy
                    )
            # kT[d, w] = Wk_eff @ xnT  (all 576 halo tokens; k-bias dropped)
            for j in range(4):
                ps = pj.tile([128, 512], f32, tag="pj")
                ps2 = pj.tile([128, 512], f32, tag="pj")
                for c in range(4):
                    nc.tensor.matmul(
                        ps[:], ws["wk"][:, c, 128 * j: 128 * (j + 1)],
                        xnT[:, c, 0:512],
                        start=(c == 0), stop=(c == 3),
                    )
                for c in range(4):
                    nc.tensor.matmul(
                        ps2[:, 0:64], ws["wk"][:, c, 128 * j: 128 * (j + 1)],
                        xnT[:, c, 512:576],
                        start=(c == 0), stop=(c == 3),
                    )
                nc.scalar.activation(out=kT[:, j, 0:512], in_=ps[:], func=AF.Copy)
                nc.scalar.activation(
                    out=kT[:, j, 512:576], in_=ps2[:, 0:64], func=AF.Copy
                )
            # v token-major (feature-permuted Wv; bias folded into bo/bg)
            for t in range(5):
                rows = 128 if t < 4 else 64
                ps = pj.tile([128, 512], f32, tag="pj")
                for c in range(4):
                    nc.tensor.matmul(
                        ps[:rows], xnT[:, c, 128 * t: 128 * t + rows],
                        ws["wv"][:, c, :],
                        start=(c == 0), stop=(c == 3),
                    )
                nc.scalar.activation(
                    out=v_sb[:rows, t, :], in_=ps[:rows], func=AF.Copy
                )

            # ---- stage 4: software-pipelined banded attention ----
            # stage st emits scores+exp for block st, then the AV/normalize
            # consumers for block st-1, then block st's masks, then block
            # st-1's epilogue.
            a_tiles = {}
            for st in range(NBLK + 1):
                if st < NBLK:
                    b = st
                    for g in range(2):
                        s0 = sp0.tile([128, 2, 256], f32, tag="s0")
                        s1 = sp1.tile([64, 2, 256], f32, tag="s1")
                        for pr in range(2):
                            p = 2 * g + pr
                            q_ap = q2[:, p, :, 128 * b: 128 * (b + 1)]
                            nc.tensor.matmul(
                                s0[:, pr, :],
                                kT[:, p, 128 * b: 128 * b + 128],
                                q_ap, start=True, stop=True,
                            )
                            nc.tensor.matmul(
                                s1[:, pr, :],
                                kT[:, p, 128 * b + 128: 128 * b + 192],
                                q_ap, start=True, stop=True,
                            )
                        a0 = attn_pool.tile([128, 2, 256], bf16, tag="a0")
                        a1 = attn_pool.tile([64, 2, 256], bf16, tag="a1")
                        # s1 first: its single PSUM buffer is the next
                        # score-matmul's dependency
                        nc.scalar.activation(
                            out=a1[:].rearrange("p a b -> p (a b)"),
                            in_=s1[:].rearrange("p a b -> p (a b)"), func=AF.Exp,
                        )
                        nc.scalar.activation(
                            out=a0[:].rearrange("p a b -> p (a b)"),
                            in_=s0[:].rearrange("p a b -> p (a b)"), func=AF.Exp,
                        )
                        a_tiles[(b, g)] = (a0, a1)
                if st >= 1:
                    bp_ = st - 1
                    cps = cp.tile([128, 4, 128], f32, tag="cps")
                    for g in range(2):
                        a0, a1 = a_tiles[(bp_, g)]
                        a0f = a0[:].rearrange("p a b -> p (a b)")
                        a1f = a1[:].rearrange("p a b -> p (a b)")
                        # denominators broadcast to every partition by an
                        # all-ones stationary
                        bc = bcp.tile([128, 512], f32, tag="bc")
                        nc.tensor.matmul(
                            bc[:], ones_sb[:], a0f, start=True, stop=False,
                        )
                        nc.tensor.matmul(
                            bc[:], ones_sb[0:64, :], a1f, start=False, stop=True,
                        )
                        # AV: V stationary -> ctx feature-major (head h in
                        # feature tile h%4, partition half h//4 = g)
                        po = 64 * g
                        for hh in range(4):
                            nc.tensor.matmul(
                                cps[po: po + 64, hh, :],
                                v_sb[:, bp_, 128 * hh + po: 128 * hh + po + 64],
                                a0[:, hh >> 1, 128 * (hh & 1): 128 * (hh & 1) + 128],
                                start=True, stop=False,
                            )
                            nc.tensor.matmul(
                                cps[po: po + 64, hh, :],
                                v_sb[0:64, bp_ + 1, 128 * hh + po: 128 * hh + po + 64],
                                a1[:, hh >> 1, 128 * (hh & 1): 128 * (hh & 1) + 128],
                                start=False, stop=True,
                            )
                        rbc = rp_pool.tile([64, 512], f32, tag="rbc")
                        nc.vector.reciprocal_approx_fast(
                            out=rbc[:], in_=bc[po: po + 64, :]
                        )
                        nc.vector.tensor_mul(
                            out=ctxT[po: po + 64, :, bp_, :],
                            in0=cps[po: po + 64, :, :],
                            in1=rbc[:].rearrange("p (a q) -> p a q", q=128),
                        )
                        del a_tiles[(bp_, g)]
                if st < NBLK:
                    b = st
                    for g in range(2):
                        a0, a1 = a_tiles[(b, g)]
                        nc.vector.tensor_mul(
                            out=a1[:].rearrange("p a b -> p (a b)"),
                            in0=a1[:].rearrange("p a b -> p (a b)"),
                            in1=mB_sb[:, b, :],
                        )
                        nc.vector.tensor_mul(
                            out=a0[:].rearrange("p a b -> p (a b)"),
                            in0=a0[:].rearrange("p a b -> p (a b)"),
                            in1=mA_sb[:, b, :],
                        )
                if st >= 1:
                    b = st - 1
                    # ---- epilogue for block b: gate (x and ctx parts share
                    # one PSUM accumulation), O-proj, tanh-gate, blend, store
                    gacc = pj.tile([128, 512], f32, tag="pj", name=f"gacc{b}")
                    for c in range(4):
                        nc.tensor.matmul(
                            gacc[:], xt_sb[:, c, 128 * b: 128 * (b + 1)],
                            ws["wg1"][:, c, :],
                            start=(c == 0), stop=False,
                        )
                    for c in range(4):
                        nc.tensor.matmul(
                            gacc[:], ctxT[:, c, b, :], ws["wg2"][:, c, :],
                            start=False, stop=(c == 3),
                        )
                    ops = pj.tile([128, 512], f32, tag="pj")
                    for c in range(4):
                        nc.tensor.matmul(
                            ops[:], ctxT[:, c, b, :], ws["wo"][:, c, :],
                            start=(c == 0), stop=(c == 3),
                        )
                    diff = fin_pool.tile([128, 512], bf16, tag="diff")
                    th = fin_pool.tile([128, 512], bf16, tag="th")
                    gate = fin_pool.tile([128, 512], bf16, tag="gate")
                    outs = fin_pool.tile([128, 512], bf16, tag="outs")
                    if bo_nz:
                        nc.vector.tensor_add(out=ops[:], in0=ops[:], in1=bo_bc[:])
                    nc.vector.tensor_sub(out=diff[:], in0=ops[:], in1=x_sb[:, b, :])
                    if bg_nz:
                        nc.vector.tensor_add(out=gacc[:], in0=gacc[:], in1=bg_bc[:])
                    # sigmoid(z) = 0.5*tanh(z/2) + 0.5  (tanh shares the exp
                    # table set -> no ACT table reload)
                    nc.scalar.activation(
                        out=th[:], in_=gacc[:], func=AF.Tanh, scale=0.5,
                    )
                    nc.vector.tensor_scalar(
                        out=gate[:], in0=th[:],
                        scalar1=0.5, scalar2=0.5, op0=ALU.mult, op1=ALU.add,
                    )
                    # out = x + gate * (o - x); bf16 chain runs DVE at 2x
                    nc.vector.tensor_mul(out=diff[:], in0=diff[:], in1=gate[:])
                    nc.vector.tensor_add(out=outs[:], in0=diff[:], in1=x_sb[:, b, :])
                    nc.sync.dma_start(
                        out=out_t[:].rearrange("(c p) d -> p c d", p=128)[:, b, :],
                        in_=outs[:],
                    )
    nc.compile()
    return nc


def _host_prep(inputs):
    """Fold LN gain/bias + scale + Wp + bv into weights, build per-core maps."""
    x = np.asarray(inputs["token_embeds"], np.float32)
    g = np.asarray(inputs["ln_g"], np.float32)
    lb = np.asarray(inputs["ln_b"], np.float32)
    Wp = np.asarray(inputs["Wp"], np.float32)
    Wq = np.asarray(inputs["Wq"], np.float32)
    Wk = np.asarray(inputs["Wk"], np.float32)
    Wv = np.asarray(inputs["Wv"], np.float32)
    Wo = np.asarray(inputs["Wo"], np.float32)
    Wg = np.asarray(inputs["Wg"], np.float32)
    bp = np.asarray(inputs["bp"], np.float32)
    bq = np.asarray(inputs["bq"], np.float32)
    bv = np.asarray(inputs["bv"], np.float32)
    bo = np.asarray(inputs["bo"], np.float32)
    bg = np.asarray(inputs["bg"], np.float32)

    scale = 1.0 / np.sqrt(np.float32(DH))
    # feature permutation for ctx: head h features -> tile h%4, half h//4
    perm = np.zeros(D, np.int64)
    for h in range(H):
        c, gg = h % 4, h // 4
        perm[128 * c + 64 * gg: 128 * c + 64 * gg + 64] = np.arange(
            64 * h, 64 * h + 64
        )

    Wpq = (Wq @ Wp) * scale                       # folded q projection
    wq = np.ascontiguousarray((Wpq * g[None, :]).T).astype(BF16)
    wk = np.ascontiguousarray((Wk * g[None, :]).T).astype(BF16)
    wv_p = (Wv * g[None, :])[perm, :]             # permuted output features
    wv = np.ascontiguousarray(wv_p.T).astype(BF16)
    wo = np.ascontiguousarray(Wo[:, perm].T).astype(BF16)
    wg1 = np.ascontiguousarray(Wg[:, :D].T).astype(BF16)
    # reference gates on ctx AFTER the O-projection; fold Wo into Wg2 so the
    # gate matmul can consume pre-projection (permuted) ctx directly
    Wg2o = Wg[:, D:] @ Wo
    wg2 = np.ascontiguousarray(Wg2o[:, perm].T).astype(BF16)

    bq_eff = (Wq @ (Wp @ lb + bp) + bq) * scale
    bv_eff = Wv @ lb + bv
    # device ctx omits the v-bias; it re-enters as a constant through both
    # the O-projection and the folded gate projection
    bo_eff = Wo @ bv_eff + bo
    bg_eff = Wg[:, D:] @ bo_eff + bg

    bqh = np.ascontiguousarray(bq_eff.reshape(4, 128).T).astype(np.float32)
    flags = (
        bool(np.any(bq_eff != 0)),
        bool(np.any(bo_eff != 0)),
        bool(np.any(bg_eff != 0)),
    )

    in_maps = []
    for core in range(NCORES):
        bi, ci = core // 4, core % 4
        s = ci * CHUNK
        xr = np.zeros((XROWS, D), BF16)
        xr[0:CHUNK] = x[bi, s: s + CHUNK]
        if s - WCTX >= 0:
            xr[CHUNK: CHUNK + WCTX] = x[bi, s - WCTX: s]
        if s + CHUNK + WCTX <= T:
            xr[CHUNK + WCTX: CHUNK + 2 * WCTX] = x[bi, s + CHUNK: s + CHUNK + WCTX]
        xt = np.ascontiguousarray(x[bi, s: s + CHUNK].T).astype(BF16)

        # mask[b, rr, cc]: query r=128b+rr (local), key halo pos j=128b+cc;
        # duplicated 4x along columns (pair x evenodd) so the on-device
        # multiply is a contiguous 2D bf16 op
        rr = np.arange(128)[:, None]
        cc = np.arange(192)[None, :]
        m = np.zeros((NBLK, 128, 192), np.float32)
        for qb in range(NBLK):
            band = (cc - rr >= 0) & (cc - rr <= 2 * WCTX)
            gkey = s + 128 * qb + cc - WCTX + 0 * rr
            m[qb] = (band & (gkey >= 0) & (gkey < T)).astype(np.float32)
        mA = np.ascontiguousarray(
            np.tile(m[:, :, :128].transpose(2, 0, 1), (1, 1, 4))
        ).astype(BF16)
        mB = np.ascontiguousarray(
            np.tile(m[:, :, 128:].transpose(2, 0, 1), (1, 1, 4))
        ).astype(BF16)

        in_maps.append({
            "x": xr, "xt": xt, "mA": mA, "mB": mB,
            "wq": wq, "wk": wk, "wv": wv, "wo": wo,
            "wg1": wg1, "wg2": wg2,
            "bqh": bqh,
            "bo": bo_eff.reshape(1, D).astype(np.float32),
            "bg": bg_eff.reshape(1, D).astype(np.float32),
        })
    return in_maps, flags


def _run(inputs, trace=False):
    from concourse.bass_utils import run_bass_kernel_spmd

    in_maps, flags = _host_prep(inputs)
    if flags not in _CACHE:
        _CACHE[flags] = _build_program(flags)
    nc = _CACHE[flags]
    res = run_bass_kernel_spmd(nc, in_maps, list(range(NCORES)), trace=trace)
    out = np.zeros((B, T, D), np.float32)
    for core in range(NCORES):
        bi, ci = core // 4, core % 4
        out[bi, ci * CHUNK: (ci + 1) * CHUNK] = np.asarray(
            res.results[core]["out"], dtype=np.float32
        )
    return out, res


def kernel(**inputs):
    out, _ = _run(inputs, trace=False)
    return out


# revision 40
# speedup vs baseline: 1.0499x; 1.0018x over previous
"""Trainium2 Bass kernel for nn_ContextEncoder (banded local attention encoder).

Reference computation (B=2, T=2048, D=512, H=8, dh=64, band half-width 32):
  xn   = LayerNorm(x) * g + b
  q    = ((xn @ Wp.T + bp) @ Wq.T + bq) / sqrt(dh)      per-head [B,T,H,dh]
  k, v = xn @ Wk.T + bk, xn @ Wv.T + bv
  s    = banded scores  (|i-j| <= 32), softmax over window
  ctx  = (a @ v_window) @ Wo.T + bo
  gate = sigmoid([x, ctx] @ Wg.T + bg)
  out  = x * (1 - gate) + ctx * gate

Sharding: sequence-parallel, 8 cores = 2 batches x 4 chunks of 512 tokens.
Each core gets its 512-token chunk plus a 32-token halo on each side
(zero-padded at sequence edges; per-core masks kill invalid positions),
computes its 512 output rows fully independently (no collectives), and the
host concatenates.

Algebraic folds done on host:
  - Wp folded into Wq:  q = xn @ (Wq Wp).T * s  -- removes a DxD projection.
  - k-bias dropped: a per-feature constant added to every key shifts each
    query's scores uniformly, which softmax cancels.
  - v-bias folded into bo/bg (ctx picks up exactly +bv after normalization).
  - LN gain/bias folded into weights; gate projections of x and ctx share
    one PSUM accumulation (gate_pre = x@Wg1.T + ctx@(Wg2 Wo).T + const).
  - gate sigmoid computed as 0.5*tanh(z/2)+0.5: tanh lives in the same ACT
    table set as exp, so the kernel performs ZERO mid-kernel table reloads
    (sqrt set loads once at t=0 via a dummy op, exp set once post-LN).

Device pipeline (per core):
  - x tile 0 DMAs first (column halves split across the sync/scalar
    queues; later hi-halves are interleaved between the LN sqrts); LN
    runs per-tile and tile t's PE transposes start as soon as LN(t)
    lands, so the PE starts right after the first LN tile instead of
    waiting for the whole LN phase.
  - Weight DMA issues: wq/wk on sync behind the small x slices, wv and
    the masks on the (slow but otherwise idle) gpsimd SW-DGE queue,
    epilogue operands (xt/wg1/wo/wg2) issued after the LN chain.
  - An ACT sqrt table preload runs at t=0 and an exp preload (pinned
    after the last LN sqrt via a data dependency) right after LN, so
    both ACT_TABLE_LOADs happen while ACT is idle.
  - Scores use the K=128 zero-padded q2 pair layout (contraction
    operands based at partition 64 fault on HW - quadrant-3 xbus).
  - exp on ACT; contiguous pre-duplicated bf16 masks multiply on DVE
    at 2x bf16 rate.
  - AV with V stationary -> ctx feature-major; heads interleaved into
    aligned PE quadrants via a host permutation of Wv/Wo/Wg2.
    Denominators via an all-ones stationary; reciprocal_approx_fast +
    multiply normalizes into the O-projection operand layout.
  - Attention is software-pipelined: block b's scores are emitted before
    block b-1's AV/epilogue so the PE queue never waits on the softmax
    chain.
  - Epilogue: gate = 0.5*tanh(gacc/2)+0.5 (ACT tanh + DVE tensor_scalar),
    bf16 blend chain (2x DVE mode), per-block bf16 stores.
"""

import numpy as np
import ml_dtypes

B, T, D = 2, 2048, 512
H, DH = 8, 64
WCTX = 32
NCORES = 8
CHUNK = 512          # tokens per core
NBLK = CHUNK // 128  # 4 query blocks per core
HALO = CHUNK + 2 * WCTX   # 576 tokens incl. halo
XROWS = 640          # x dram rows: 512 central + 32 left + 32 right + 64 pad
BF16 = ml_dtypes.bfloat16

_CACHE = {}


def _build_program(flags):
    """Builds the single-core Bass/Tile program (shared SPMD across 8 cores).

    flags: (bq_nz, bo_nz, bg_nz) -> emit optional bias adds.
    """
    import concourse.bass as bass
    import concourse.tile as tile
    import concourse.mybir as mybir
    from concourse import bacc

    f32 = mybir.dt.float32
    bf16 = mybir.dt.bfloat16
    AF = mybir.ActivationFunctionType
    ALU = mybir.AluOpType
    bq_nz, bo_nz, bg_nz = flags

    nc = bacc.Bacc(
        "TRN2",
        target_bir_lowering=False,
        debug=False,
        enable_asserts=False,
        num_devices=NCORES,
    )

    x_in = nc.dram_tensor("x", [XROWS, D], bf16, kind="ExternalInput")
    xt_in = nc.dram_tensor("xt", [D, CHUNK], bf16, kind="ExternalInput")
    mA_in = nc.dram_tensor("mA", [128, NBLK, 512], bf16, kind="ExternalInput")
    mB_in = nc.dram_tensor("mB", [64, NBLK, 512], bf16, kind="ExternalInput")
    w_in = {
        n: nc.dram_tensor(n, [D, D], bf16, kind="ExternalInput")
        for n in ["wq", "wk", "wv", "wo", "wg1", "wg2"]
    }
    bqh_in = nc.dram_tensor("bqh", [128, 4], f32, kind="ExternalInput")
    bo_in = nc.dram_tensor("bo", [1, D], f32, kind="ExternalInput")
    bg_in = nc.dram_tensor("bg", [1, D], f32, kind="ExternalInput")
    out_t = nc.dram_tensor("out", [CHUNK, D], bf16, kind="ExternalOutput")

    with tile.TileContext(nc) as tc:
        with (
            tc.tile_pool(name="wpool", bufs=1) as wpool,
            tc.tile_pool(name="apool", bufs=1) as apool,
            tc.tile_pool(name="small", bufs=1) as small,
            tc.tile_pool(name="stats", bufs=12) as stats_pool,
            tc.tile_pool(name="attn", bufs=6) as attn_pool,
            tc.tile_pool(name="rp", bufs=3) as rp_pool,
            tc.tile_pool(name="fin", bufs=3) as fin_pool,
            tc.tile_pool(name="pj", bufs=3, space="PSUM") as pj,
            tc.tile_pool(name="sp0", bufs=2, space="PSUM") as sp0,
            tc.tile_pool(name="sp1", bufs=1, space="PSUM") as sp1,
            tc.tile_pool(name="cp", bufs=1, space="PSUM") as cp,
            tc.tile_pool(name="bcp", bufs=1, space="PSUM") as bcp,
        ):
            # ---- persistent SBUF tensors ----
            x_sb = apool.tile([128, 5, D], bf16, tag="x")
            xn0 = apool.tile([128, 5, D], bf16, tag="xn0")
            xnT = apool.tile([128, 4, HALO], bf16, tag="xnT")
            q2 = apool.tile([128, 4, 2, CHUNK], bf16, tag="q2")
            kT = apool.tile([128, 4, HALO], bf16, tag="kT")
            v_sb = apool.tile([128, 5, D], bf16, tag="v")
            xt_sb = apool.tile([128, 4, CHUNK], bf16, tag="xt")
            mA_sb = apool.tile([128, NBLK, 512], bf16, tag="mA")
            mB_sb = apool.tile([64, NBLK, 512], bf16, tag="mB")
            ctxT = apool.tile([128, 4, NBLK, 128], bf16, tag="ctxT")

            ws = {n: wpool.tile([128, 4, D], bf16, tag=n, name=n) for n in w_in}
            eps_t = small.tile([128, 1], f32, tag="eps")
            preld = small.tile([128, 1], f32, tag="preld")
            ones_sb = small.tile([128, 128], bf16, tag="ones")
            ident = small.tile([128, 128], bf16, tag="ident")

            def wdma(eng, name):
                eng.dma_start(
                    out=ws[name][:],
                    in_=w_in[name][:].rearrange("(c p) d -> p c d", p=128),
                )

            # ---- input DMAs ----
            # x tile 0 goes first, split over two queues, so LN(t0) (and
            # with it the first PE transpose) starts ASAP.  The remaining
            # hi-halves are interleaved between the LN sqrts below so the
            # scalar queue never delays the sqrt chain.
            x_view = x_in[:].rearrange("(c p) d -> p c d", p=128)
            nc.sync.dma_start(out=x_sb[:, 0:1, 0:256], in_=x_view[:, 0:1, 0:256])
            nc.scalar.dma_start(out=x_sb[:, 0:1, 256:512], in_=x_view[:, 0:1, 256:512])
            nc.vector.memset(preld[:], 1.0)
            nc.vector.memset(eps_t[:], 1e-5)
            # sqrt table preload: ACT is idle while x lands; walrus puts the
            # sqrt-set ACT_TABLE_LOAD right before this op, off-critical.
            nc.scalar.activation(out=preld[:], in_=preld[:], func=AF.Sqrt)
            for sl in range(1, 5):
                nc.scalar.dma_start(
                    out=x_sb[:, sl: sl + 1, 256:512],
                    in_=x_view[:, sl: sl + 1, 256:512],
                )
            for sl in range(1, 5):
                nc.sync.dma_start(
                    out=x_sb[:, sl: sl + 1, 0:256],
                    in_=x_view[:, sl: sl + 1, 0:256],
                )
            from concourse.masks import make_identity
            make_identity(nc, ident[:])
            nc.gpsimd.memset(ones_sb[:], 1.0)
            # zero the dead half of each q2 (pair, evenodd) slot once.
            # (contraction operands based at partition 64 fault on HW —
            # quadrant-3 xbus — so scores use the K=128 zero-padded layout)
            for p in range(4):
                nc.gpsimd.memset(q2[64:128, p, 0, :], 0.0)
                nc.gpsimd.memset(q2[0:64, p, 1, :], 0.0)
            # weights: wq/wk on sync after the tiny x-lo issues; wv + masks
            # ride the (slow but idle) gpsimd SW-DGE queue.
            wdma(nc.sync, "wq")
            wdma(nc.sync, "wk")
            wdma(nc.gpsimd, "wv")
            nc.gpsimd.dma_start(out=mA_sb[:], in_=mA_in[:])
            nc.gpsimd.dma_start(out=mB_sb[:], in_=mB_in[:])

            bqh = bo_bc = bg_bc = None
            if bq_nz:
                bqh = small.tile([128, 4], f32, tag="bqh")
                nc.gpsimd.dma_start(out=bqh[:], in_=bqh_in[:])
            if bo_nz:
                bo_bc = small.tile([128, D], f32, tag="bo_bc")
                nc.gpsimd.dma_start(out=bo_bc[:], in_=bo_in[:].to_broadcast([128, D]))
            if bg_nz:
                bg_bc = small.tile([128, D], f32, tag="bg_bc")
                nc.gpsimd.dma_start(out=bg_bc[:], in_=bg_in[:].to_broadcast([128, D]))

            # ---- stage 1+2 fused: software-pipelined LayerNorm -> PE
            # transposes.  The DVE queue is in-order, so tile t+1's
            # bn_stats/bn_aggr are emitted BEFORE tile t's recip/normalize:
            # otherwise the normalize (which waits on the ACT sqrt round
            # trip) blocks the next tile's stats and the PE starves.
            rstd4 = None
            ln_state = {}

            def ln_front(t):
                rows = 128 if t < 4 else 64
                st = stats_pool.tile([128, 6], f32, tag="st")
                mv = stats_pool.tile([128, 2], f32, tag="mv")
                rstd = stats_pool.tile([128, 1], f32, tag="rstd")
                nc.vector.bn_stats(out=st[:rows], in_=x_sb[:rows, t, :])
                nc.vector.bn_aggr(out=mv[:rows], in_=st[:rows])
                nc.scalar.activation(
                    out=rstd[:rows], in_=mv[:rows, 1:2], func=AF.Sqrt,
                    bias=eps_t[:rows], scale=1.0,
                )
                ln_state[t] = (mv, rstd, rows)

            def ln_back(t):
                mv, rstd, rows = ln_state[t]
                nc.vector.reciprocal(out=rstd[:rows], in_=rstd[:rows])
                nc.vector.tensor_scalar(
                    out=xn0[:rows, t, :], in0=x_sb[:rows, t, :],
                    scalar1=mv[:rows, 0:1], scalar2=rstd[:rows],
                    op0=ALU.subtract, op1=ALU.mult,
                )
                if t < 4:
                    tp = pj.tile([128, 4, 128], bf16, tag="pj", name=f"tp{t}")
                    for j in range(4):
                        nc.tensor.transpose(
                            tp[:, j, :],
                            xn0[:, t, 128 * j: 128 * (j + 1)],
                            ident[:],
                        )
                    nc.vector.tensor_copy(
                        out=xnT[:, :, 32 + 128 * t: 32 + 128 * (t + 1)],
                        in_=tp[:],
                    )
                else:
                    # halo tokens: x rows [512:544] -> halo 0..32,
                    # [544:576] -> halo 544..576
                    tp = pj.tile([128, 4, 128], bf16, tag="pj", name="tph")
                    for j in range(4):
                        nc.tensor.transpose(
                            tp[:, j, 0:64],
                            xn0[0:64, 4, 128 * j: 128 * (j + 1)],
                            ident[0:64, 0:64],
                        )
                    nc.vector.tensor_copy(
                        out=xnT[:, :, 0:32], in_=tp[:, :, 0:32]
                    )
                    nc.vector.tensor_copy(
                        out=xnT[:, :, 544:576], in_=tp[:, :, 32:64]
                    )

            for t in [0, 1, 2, 3, 4]:
                ln_front(t)
                if t >= 1:
                    ln_back(t - 1)
            ln_back(4)
            rstd4 = ln_state[4][1]

            # exp/tanh table preload on the now-idle ACT; the rstd4 input
            # pins it AFTER the last LN sqrt in the ACT stream.  All later
            # ACT ops (Copy, Exp, Tanh) live in this one set -> no reloads.
            nc.scalar.activation(out=preld[0:1], in_=rstd4[0:1], func=AF.Exp, scale=0.0)

            # late inputs: epilogue operands, issued after the LN chain
            nc.scalar.dma_start(
                out=xt_sb[:], in_=xt_in[:].rearrange("(c p) d -> p c d", p=128)
            )
            wdma(nc.scalar, "wg1")
            wdma(nc.sync, "wo")
            wdma(nc.sync, "wg2")

            # ---- stage 3: projections ----
            # q: folded single projection, written into the zero-padded pair
            # layout (head-even rows -> partitions 0-63, head-odd -> 64-127)
            for j in range(4):
                ps = pj.tile([128, 512], f32, tag="pj")
                for c in range(4):
                    nc.tensor.matmul(
                        ps[:], ws["wq"][:, c, 128 * j: 128 * (j + 1)],
                        xnT[:, c, 32: 32 + CHUNK],
                        start=(c == 0), stop=(c == 3),
                    )
                if bq_nz:
                    nc.vector.tensor_scalar(
                        out=q2[0:64, j, 0, :], in0=ps[0:64],
                        scalar1=bqh[0:64, j: j + 1], scalar2=None, op0=ALU.add,
                    )
                    nc.vector.tensor_scalar(
                        out=q2[64:128, j, 1, :], in0=ps[64:128],
                        scalar1=bqh[64:128, j: j + 1], scalar2=None, op0=ALU.add,
                    )
                else:
                    # DVE copies: ACT otherwise backlogs 8 q-copies ahead
                    # of the kT copies, stalling k's PSUM ring
                    nc.vector.tensor_copy(out=q2[0:64, j, 0, :], in_=ps[0:64])
                    nc.vector.tensor_copy(out=q2[64:128, j, 1, :], in_=ps[64:128])
            # kT[d, w] = Wk_eff @ xnT  (all 576 halo tokens; k-bias dropped)
            for j in range(4):
                ps = pj.tile([128, 512], f32, tag="pj")
                ps2 = pj.tile([128, 512], f32, tag="pj")
                for c in range(4):
                    nc.tensor.matmul(
                        ps[:], ws["wk"][:, c, 128 * j: 128 * (j + 1)],
                        xnT[:, c, 0:512],
                        start=(c == 0), stop=(c == 3),
                    )
                for c in range(4):
                    nc.tensor.matmul(
                        ps2[:, 0:64], ws["wk"][:, c, 128 * j: 128 * (j + 1)],
                        xnT[:, c, 512:576],
                        start=(c == 0), stop=(c == 3),
                    )
                nc.scalar.activation(out=kT[:, j, 0:512], in_=ps[:], func=AF.Copy)
                nc.scalar.activation(
                    out=kT[:, j, 512:576], in_=ps2[:, 0:64], func=AF.Copy
                )
            # v token-major (feature-permuted Wv; bias folded into bo/bg)
            for t in range(5):
                rows = 128 if t < 4 else 64
                ps = pj.tile([128, 512], f32, tag="pj")
                for c in range(4):
                    nc.tensor.matmul(
                        ps[:rows], xnT[:, c, 128 * t: 128 * t + rows],
                        ws["wv"][:, c, :],
                        start=(c == 0), stop=(c == 3),
                    )
                nc.scalar.activation(
                    out=v_sb[:rows, t, :], in_=ps[:rows], func=AF.Copy
                )

            # ---- stage 4: software-pipelined banded attention ----
            # stage st emits scores+exp for block st, then the AV/normalize
            # consumers for block st-1, then block st's masks, then block
            # st-1's epilogue.
            a_tiles = {}
            for st in range(NBLK + 1):
                if st < NBLK:
                    b = st
                    for g in range(2):
                        s0 = sp0.tile([128, 2, 256], f32, tag="s0")
                        s1 = sp1.tile([64, 2, 256], f32, tag="s1")
                        for pr in range(2):
                            p = 2 * g + pr
                            q_ap = q2[:, p, :, 128 * b: 128 * (b + 1)]
                            nc.tensor.matmul(
                                s0[:, pr, :],
                                kT[:, p, 128 * b: 128 * b + 128],
                                q_ap, start=True, stop=True,
                            )
                            nc.tensor.matmul(
                                s1[:, pr, :],
                                kT[:, p, 128 * b + 128: 128 * b + 192],
                                q_ap, start=True, stop=True,
                            )
                        a0 = attn_pool.tile([128, 2, 256], bf16, tag="a0")
                        a1 = attn_pool.tile([64, 2, 256], bf16, tag="a1")
                        # s1 first: its single PSUM buffer is the next
                        # score-matmul's dependency
                        nc.scalar.activation(
                            out=a1[:].rearrange("p a b -> p (a b)"),
                            in_=s1[:].rearrange("p a b -> p (a b)"), func=AF.Exp,
                        )
                        nc.scalar.activation(
                            out=a0[:].rearrange("p a b -> p (a b)"),
                            in_=s0[:].rearrange("p a b -> p (a b)"), func=AF.Exp,
                        )
                        a_tiles[(b, g)] = (a0, a1)
                if st >= 1:
                    bp_ = st - 1
                    cps = cp.tile([128, 4, 128], f32, tag="cps")
                    for g in range(2):
                        a0, a1 = a_tiles[(bp_, g)]
                        a0f = a0[:].rearrange("p a b -> p (a b)")
                        a1f = a1[:].rearrange("p a b -> p (a b)")
                        # denominators broadcast to every partition by an
                        # all-ones stationary
                        bc = bcp.tile([128, 512], f32, tag="bc")
                        nc.tensor.matmul(
                            bc[:], ones_sb[:], a0f, start=True, stop=False,
                        )
                        nc.tensor.matmul(
                            bc[:], ones_sb[0:64, :], a1f, start=False, stop=True,
                        )
                        # AV: V stationary -> ctx feature-major (head h in
                        # feature tile h%4, partition half h//4 = g)
                        po = 64 * g
                        for hh in range(4):
                            nc.tensor.matmul(
                                cps[po: po + 64, hh, :],
                                v_sb[:, bp_, 128 * hh + po: 128 * hh + po + 64],
                                a0[:, hh >> 1, 128 * (hh & 1): 128 * (hh & 1) + 128],
                                start=True, stop=False,
                            )
                            nc.tensor.matmul(
                                cps[po: po + 64, hh, :],
                                v_sb[0:64, bp_ + 1, 128 * hh + po: 128 * hh + po + 64],
                                a1[:, hh >> 1, 128 * (hh & 1): 128 * (hh & 1) + 128],
                                start=False, stop=True,
                            )
                        rbc = rp_pool.tile([64, 512], f32, tag="rbc")
                        nc.vector.reciprocal_approx_fast(
                            out=rbc[:], in_=bc[po: po + 64, :]
                        )
                        nc.vector.tensor_mul(
                            out=ctxT[po: po + 64, :, bp_, :],
                            in0=cps[po: po + 64, :, :],
                            in1=rbc[:].rearrange("p (a q) -> p a q", q=128),
                        )
                        del a_tiles[(bp_, g)]
                if st < NBLK:
                    b = st
                    for g in range(2):
                        a0, a1 = a_tiles[(b, g)]
                        nc.vector.tensor_mul(
                            out=a1[:].rearrange("p a b -> p (a b)"),
                            in0=a1[:].rearrange("p a b -> p (a b)"),
                            in1=mB_sb[:, b, :],
                        )
                        nc.vector.tensor_mul(
                            out=a0[:].rearrange("p a b -> p (a b)"),
                            in0=a0[:].rearrange("p a b -> p (a b)"),
                            in1=mA_sb[:, b, :],
                        )
                if st >= 1:
                    b = st - 1
                    # ---- epilogue for block b: gate (x and ctx parts share
                    # one PSUM accumulation), O-proj, tanh-gate, blend, store
                    gacc = pj.tile([128, 512], f32, tag="pj", name=f"gacc{b}")
                    for c in range(4):
                        nc.tensor.matmul(
                            gacc[:], xt_sb[:, c, 128 * b: 128 * (b + 1)],
                            ws["wg1"][:, c, :],
                            start=(c == 0), stop=False,
                        )
                    for c in range(4):
                        nc.tensor.matmul(
                            gacc[:], ctxT[:, c, b, :], ws["wg2"][:, c, :],
                            start=False, stop=(c == 3),
                        )
                    ops = pj.tile([128, 512], f32, tag="pj")
                    for c in range(4):
                        nc.tensor.matmul(
                            ops[:], ctxT[:, c, b, :], ws["wo"][:, c, :],
                            start=(c == 0), stop=(c == 3),
                        )
                    diff = fin_pool.tile([128, 512], bf16, tag="diff")
                    th = fin_pool.tile([128, 512], bf16, tag="th")
                    gate = fin_pool.tile([128, 512], bf16, tag="gate")
                    outs = fin_pool.tile([128, 512], bf16, tag="outs")
                    if bo_nz:
                        nc.vector.tensor_add(out=ops[:], in0=ops[:], in1=bo_bc[:])
                    nc.vector.tensor_sub(out=diff[:], in0=ops[:], in1=x_sb[:, b, :])
                    if bg_nz:
                        nc.vector.tensor_add(out=gacc[:], in0=gacc[:], in1=bg_bc[:])
                    # sigmoid(z) = 0.5*tanh(z/2) + 0.5  (tanh shares the exp
                    # table set -> no ACT table reload)
                    nc.scalar.activation(
                        out=th[:], in_=gacc[:], func=AF.Tanh, scale=0.5,
                    )
                    nc.vector.tensor_scalar(
                        out=gate[:], in0=th[:],
                        scalar1=0.5, scalar2=0.5, op0=ALU.mult, op1=ALU.add,
                    )
                    # out = x + gate * (o - x); bf16 chain runs DVE at 2x
                    nc.vector.tensor_mul(out=diff[:], in0=diff[:], in1=gate[:])
                    nc.vector.tensor_add(out=outs[:], in0=diff[:], in1=x_sb[:, b, :])
                    nc.sync.dma_start(
                        out=out_t[:].rearrange("(c p) d -> p c d", p=128)[:, b, :],
                        in_=outs[:],
                    )
    nc.compile()
    return nc


def _host_prep(inputs):
    """Fold LN gain/bias + scale + Wp + bv into weights, build per-core maps."""
    x = np.asarray(inputs["token_embeds"], np.float32)
    g = np.asarray(inputs["ln_g"], np.float32)
    lb = np.asarray(inputs["ln_b"], np.float32)
    Wp = np.asarray(inputs["Wp"], np.float32)
    Wq = np.asarray(inputs["Wq"], np.float32)
    Wk = np.asarray(inputs["Wk"], np.float32)
    Wv = np.asarray(inputs["Wv"], np.float32)
    Wo = np.asarray(inputs["Wo"], np.float32)
    Wg = np.asarray(inputs["Wg"], np.float32)
    bp = np.asarray(inputs["bp"], np.float32)
    bq = np.asarray(inputs["bq"], np.float32)
    bv = np.asarray(inputs["bv"], np.float32)
    bo = np.asarray(inputs["bo"], np.float32)
    bg = np.asarray(inputs["bg"], np.float32)

    scale = 1.0 / np.sqrt(np.float32(DH))
    # feature permutation for ctx: head h features -> tile h%4, half h//4
    perm = np.zeros(D, np.int64)
    for h in range(H):
        c, gg = h % 4, h // 4
        perm[128 * c + 64 * gg: 128 * c + 64 * gg + 64] = np.arange(
            64 * h, 64 * h + 64
        )

    Wpq = (Wq @ Wp) * scale                       # folded q projection
    wq = np.ascontiguousarray((Wpq * g[None, :]).T).astype(BF16)
    wk = np.ascontiguousarray((Wk * g[None, :]).T).astype(BF16)
    wv_p = (Wv * g[None, :])[perm, :]             # permuted output features
    wv = np.ascontiguousarray(wv_p.T).astype(BF16)
    wo = np.ascontiguousarray(Wo[:, perm].T).astype(BF16)
    wg1 = np.ascontiguousarray(Wg[:, :D].T).astype(BF16)
    # reference gates on ctx AFTER the O-projection; fold Wo into Wg2 so the
    # gate matmul can consume pre-projection (permuted) ctx directly
    Wg2o = Wg[:, D:] @ Wo
    wg2 = np.ascontiguousarray(Wg2o[:, perm].T).astype(BF16)

    bq_eff = (Wq @ (Wp @ lb + bp) + bq) * scale
    bv_eff = Wv @ lb + bv
    # device ctx omits the v-bias; it re-enters as a constant through both
    # the O-projection and the folded gate projection
    bo_eff = Wo @ bv_eff + bo
    bg_eff = Wg[:, D:] @ bo_eff + bg

    bqh = np.ascontiguousarray(bq_eff.reshape(4, 128).T).astype(np.float32)
    flags = (
        bool(np.any(bq_eff != 0)),
        bool(np.any(bo_eff != 0)),
        bool(np.any(bg_eff != 0)),
    )

    in_maps = []
    for core in range(NCORES):
        bi, ci = core // 4, core % 4
        s = ci * CHUNK
        xr = np.zeros((XROWS, D), BF16)
        xr[0:CHUNK] = x[bi, s: s + CHUNK]
        if s - WCTX >= 0:
            xr[CHUNK: CHUNK + WCTX] = x[bi, s - WCTX: s]
        if s + CHUNK + WCTX <= T:
            xr[CHUNK + WCTX: CHUNK + 2 * WCTX] = x[bi, s + CHUNK: s + CHUNK + WCTX]
        xt = np.ascontiguousarray(x[bi, s: s + CHUNK].T).astype(BF16)

        # mask[b, rr, cc]: query r=128b+rr (local), key halo pos j=128b+cc;
        # duplicated 4x along columns (pair x evenodd) so the on-device
        # multiply is a contiguous 2D bf16 op
        rr = np.arange(128)[:, None]
        cc = np.arange(192)[None, :]
        m = np.zeros((NBLK, 128, 192), np.float32)
        for qb in range(NBLK):
            band = (cc - rr >= 0) & (cc - rr <= 2 * WCTX)
            gkey = s + 128 * qb + cc - WCTX + 0 * rr
            m[qb] = (band & (gkey >= 0) & (gkey < T)).astype(np.float32)
        mA = np.ascontiguousarray(
            np.tile(m[:, :, :128].transpose(2, 0, 1), (1, 1, 4))
        ).astype(BF16)
        mB = np.ascontiguousarray(
            np.tile(m[:, :, 128:].transpose(2, 0, 1), (1, 1, 4))
        ).astype(BF16)

        in_maps.append({
            "x": xr, "xt": xt, "mA": mA, "mB": mB,
            "wq": wq, "wk": wk, "wv": wv, "wo": wo,
            "wg1": wg1, "wg2": wg2,
            "bqh": bqh,
            "bo": bo_eff.reshape(1, D).astype(np.float32),
            "bg": bg_eff.reshape(1, D).astype(np.float32),
        })
    return in_maps, flags


def _run(inputs, trace=False):
    from concourse.bass_utils import run_bass_kernel_spmd

    in_maps, flags = _host_prep(inputs)
    if flags not in _CACHE:
        _CACHE[flags] = _build_program(flags)
    nc = _CACHE[flags]
    res = run_bass_kernel_spmd(nc, in_maps, list(range(NCORES)), trace=trace)
    out = np.zeros((B, T, D), np.float32)
    for core in range(NCORES):
        bi, ci = core // 4, core % 4
        out[bi, ci * CHUNK: (ci + 1) * CHUNK] = np.asarray(
            res.results[core]["out"], dtype=np.float32
        )
    return out, res


def kernel(**inputs):
    out, _ = _run(inputs, trace=False)
    return out
